# revision 5
# baseline (speedup 1.0000x reference)
"""AMGCN (3-layer GCN + global mean pool) distributed Bass kernel, 8 TRN2 cores.

Sharding: nodes split contiguously across 8 cores (12500 each, padded to 12544).
Per layer: local dense Z = H @ W (bf16), AllGather -> full Z table in every HBM,
then per-core aggregation over its dst-partitioned edges via dma_gather (int16
chunk-relative indices) + one-hot scatter matmuls accumulating in PSUM/SBUF.
Global mean pool via batch-id one-hot matmuls + AllReduce of the [512] readout.

The instruction stream is identical on all 8 cores (SPMD); all per-core
variation lives in input tensors (idx / dstoff / norm / batch one-hot data).
"""

import sys

if "/opt/trn_rl_repo" not in sys.path:
    sys.path.insert(0, "/opt/trn_rl_repo")

import numpy as np

P = 128
N_CORES = 8
N_NODES = 100000
N_GRAPHS = 512
D_IN = 300
D_HID = 300
D_OUT = 128
NPC = N_NODES // N_CORES          # 12500 real nodes per core
NBLK = 98                         # dst blocks per core
NPC_PAD = NBLK * P                # 12544 padded local slots
V_PAD = NPC_PAD * N_CORES         # 100352 padded global rows
N_CHUNKS = 4
CHUNK = V_PAD // N_CHUNKS         # 25088 (int16-safe) rows per gather chunk
SG_BLOCKS = 12                    # dst blocks per supergroup
D_PAD = 384                       # padded feature width (768B bf16, %256==0)
MAX_T = 8                         # tiles per dma_gather (1024 idx ring limit)


# ---------------------------------------------------------------------------
# host-side preprocessing (pure index manipulation)
# ---------------------------------------------------------------------------

def preprocess(edge_index, batch):
    src = np.asarray(edge_index[0], np.int64)
    dst = np.asarray(edge_index[1], np.int64)
    batch = np.asarray(batch, np.int64)
    loops = np.arange(N_NODES, dtype=np.int64)
    src = np.concatenate([src, loops])
    dst = np.concatenate([dst, loops])
    deg = np.bincount(dst, minlength=N_NODES).astype(np.float32)
    dinv = np.where(deg > 0, 1.0 / np.sqrt(deg), 0.0).astype(np.float32)
    norm = (dinv[src] * dinv[dst]).astype(np.float32)
    src_pad = (src // NPC) * NPC_PAD + (src % NPC)

    n_sg = (NBLK + SG_BLOCKS - 1) // SG_BLOCKS
    per_core = []
    for c in range(N_CORES):
        lo = c * NPC
        sel = (dst >= lo) & (dst < lo + NPC)
        s = src_pad[sel]
        d = dst[sel] - lo
        w = norm[sel]
        blk = d // P
        ch = s // CHUNK
        sg = blk // SG_BLOCKS
        order = np.lexsort((blk, ch, sg))
        per_core.append((s[order], d[order], w[order],
                         blk[order], ch[order], sg[order]))

    # common segment sizes: for every (sg, chunk, block), tiles = max over cores
    seg_n = {}
    core_groups = []
    for s, d, w, blk, ch, sg in per_core:
        key = sg * (N_CHUNKS * NBLK) + ch * NBLK + blk
        bounds = np.flatnonzero(np.diff(key)) + 1
        starts = np.concatenate([[0], bounds]).astype(np.int64)
        ends = np.concatenate([bounds, [len(key)]]).astype(np.int64)
        groups = {}
        for a, b in zip(starts, ends):
            k = (int(sg[a]), int(ch[a]), int(blk[a]))
            groups[k] = (a, b)
            nt = (b - a + P - 1) // P
            seg_n[k] = max(seg_n.get(k, 0), int(nt))
        core_groups.append(groups)

    order_keys = sorted(seg_n.keys())
    tile_lo = {}
    pos = 0
    for k in order_keys:
        tile_lo[k] = pos
        pos += seg_n[k]
    total_tiles = pos
    n_e = total_tiles * P

    # canonical schedule
    sg_segments = [[] for _ in range(n_sg)]
    gathers = []
    for g in range(n_sg):
        keys = [k for k in order_keys if k[0] == g]
        for k in keys:
            sg_segments[g].append((k[1], k[2], tile_lo[k], seg_n[k]))
        i = 0
        while i < len(keys):
            chv = keys[i][1]
            t0 = tile_lo[keys[i]]
            t_end = t0
            j = i
            while j < len(keys) and keys[j][1] == chv:
                t_end = tile_lo[keys[j]] + seg_n[keys[j]]
                j += 1
            t = t0
            while t < t_end:
                n_t = min(MAX_T, t_end - t)
                gathers.append((chv, t, n_t))
                t += n_t
            i = j

    # per-core packed arrays
    core_meta = []
    for cidx, (s, d, w, blk, ch, sg) in enumerate(per_core):
        idx_flat = np.zeros(n_e, np.int64)
        off_flat = np.zeros(n_e, np.int64)
        nrm_flat = np.zeros(n_e, np.float32)
        for k, (a, b) in core_groups[cidx].items():
            p0 = tile_lo[k] * P
            n = b - a
            idx_flat[p0:p0 + n] = s[a:b] % CHUNK
            off_flat[p0:p0 + n] = d[a:b] % P
            nrm_flat[p0:p0 + n] = w[a:b]
        idx_pack = np.zeros((P, n_e // 16), np.int16)
        ar = np.arange(n_e)
        idx_pack[ar % 16, ar // 16] = idx_flat.astype(np.int16)
        idx_pack[16:32] = idx_pack[0:16]
        dstoff = off_flat.reshape(total_tiles, P).T.astype(np.float32)
        nrm = nrm_flat.reshape(total_tiles, P).T.astype(np.float32)

        lo = cidx * NPC
        bf = np.full(NPC_PAD, -1.0, np.float32)
        bf[:NPC] = batch[lo:lo + NPC].astype(np.float32)
        batchf = bf.reshape(NBLK, P).T.astype(np.float32)  # [128, 98]
        core_meta.append(dict(idx=idx_pack, dstoff=dstoff, norm=nrm,
                              batchf=np.ascontiguousarray(batchf)))

    cnt = np.bincount(batch, minlength=N_GRAPHS).astype(np.float32)
    invcnt = (1.0 / np.maximum(cnt, 1.0)).astype(np.float32)
    return core_meta, gathers, sg_segments, total_tiles, invcnt


# ---------------------------------------------------------------------------
# device graph
# ---------------------------------------------------------------------------

def build_graph(gathers, sg_segments, total_tiles):
    from concourse import bass, bacc, mybir, tile
    from concourse.masks import make_identity

    f32, bf16 = mybir.dt.float32, mybir.dt.bfloat16
    i16, i32 = mybir.dt.int16, mybir.dt.int32
    CW = [P, P, D_HID - 2 * P]
    n_sg = len(sg_segments)

    nc = bacc.Bacc("TRN2", target_bir_lowering=False, debug=False,
                   num_devices=N_CORES)

    x_in = nc.dram_tensor("x", [NPC, D_IN], f32, kind="ExternalInput")
    W_in = {n: nc.dram_tensor(n, s, f32, kind="ExternalInput")
            for n, s in (("W1", [D_IN, D_HID]), ("W2", [D_HID, D_HID]),
                         ("W3", [D_HID, D_OUT]))}
    b_in = {n: nc.dram_tensor(n, s, f32, kind="ExternalInput")
            for n, s in (("b1", [D_HID]), ("b2", [D_HID]), ("b3", [D_OUT]))}
    Wm_in = nc.dram_tensor("Wm", [D_OUT, 1], f32, kind="ExternalInput")
    bm_in = nc.dram_tensor("bm", [1], f32, kind="ExternalInput")
    idx_in = nc.dram_tensor("m_idx", [P, total_tiles * 8], i16, kind="ExternalInput")
    doff_in = nc.dram_tensor("m_dstoff", [P, total_tiles], f32, kind="ExternalInput")
    norm_in = nc.dram_tensor("m_norm", [P, total_tiles], f32, kind="ExternalInput")
    batf_in = nc.dram_tensor("m_batchf", [P, NBLK], f32, kind="ExternalInput")
    invc_in = nc.dram_tensor("m_invcnt", [N_GRAPHS], f32, kind="ExternalInput")
    out_ext = nc.dram_tensor("out", [N_GRAPHS], f32, kind="ExternalOutput")

    z_shard = {1: nc.dram_tensor("z1s", [NPC_PAD, D_PAD], bf16),
               2: nc.dram_tensor("z2s", [NPC_PAD, D_PAD], bf16),
               3: nc.dram_tensor("z3s", [NPC_PAD, D_OUT], bf16)}
    z_full = {1: nc.dram_tensor("z1f", [V_PAD, D_PAD], bf16, addr_space="Shared"),
              2: nc.dram_tensor("z2f", [V_PAD, D_PAD], bf16, addr_space="Shared"),
              3: nc.dram_tensor("z3f", [V_PAD, D_OUT], bf16, addr_space="Shared")}
    ar_in = nc.dram_tensor("ar_in", [1, N_GRAPHS], f32)
    ar_out = nc.dram_tensor("ar_out", [1, N_GRAPHS], f32, addr_space="Shared")

    with tile.TileContext(nc) as tc:
        with (
            tc.tile_pool(name="const", bufs=1) as const,
            tc.tile_pool(name="sbuf", bufs=4) as sbuf,
            tc.tile_pool(name="accp", bufs=2 * SG_BLOCKS + 2) as accp,
            tc.tile_pool(name="psum", bufs=2, space="PSUM") as psum,
            tc.tile_pool(name="psz", bufs=2, space="PSUM") as psz,
            tc.tile_pool(name="psp", bufs=4, space="PSUM") as psp,
        ):
            # ---- constants ------------------------------------------------
            iotas = []
            for k in range(4):
                ii = const.tile([P, P], i32, name=f"iota_i{k}")
                nc.gpsimd.iota(ii[:], pattern=[[1, P]], base=128 * k,
                               channel_multiplier=0)
                fo = const.tile([P, P], f32, name=f"iota_f{k}")
                nc.vector.tensor_copy(out=fo[:], in_=ii[:])
                iotas.append(fo)
            ident = const.tile([P, P], bf16)
            make_identity(nc, ident[:])
            ones1 = const.tile([1, P], bf16)
            nc.vector.memset(ones1[:], 1.0)

            idx_sb = const.tile([P, total_tiles * 8], i16)
            nc.sync.dma_start(out=idx_sb[:], in_=idx_in[:, :])
            doff_sb = const.tile([P, total_tiles], f32)
            nc.sync.dma_start(out=doff_sb[:], in_=doff_in[:, :])
            norm_sb = const.tile([P, total_tiles], f32)
            nc.sync.dma_start(out=norm_sb[:], in_=norm_in[:, :])
            batf_sb = const.tile([P, NBLK], f32)
            nc.sync.dma_start(out=batf_sb[:], in_=batf_in[:, :])

            w_sb = {}
            for name, dcols in (("W1", D_HID), ("W2", D_HID), ("W3", D_OUT)):
                chunks = []
                for c in range(3):
                    wc = CW[c]
                    tf = sbuf.tile([P, dcols], f32, tag="wtmp", name=f"wt{name}{c}")
                    nc.sync.dma_start(out=tf[0:wc, :],
                                      in_=W_in[name][c * P:c * P + wc, :])
                    tb = const.tile([P, dcols], bf16, name=f"w_{name}_{c}")
                    nc.vector.tensor_copy(out=tb[0:wc, :], in_=tf[0:wc, :])
                    chunks.append(tb)
                w_sb[name] = chunks

            # bias broadcast tiles: B[n, f] = b[f] for all n (K=1 matmul)
            b_bc = {}
            for name, dcols in (("b1", D_HID), ("b2", D_HID), ("b3", D_OUT)):
                br_f = sbuf.tile([1, dcols], f32, tag="wtmp", name=f"brf_{name}")
                nc.sync.dma_start(out=br_f[:], in_=b_in[name][None, :])
                br = sbuf.tile([1, dcols], bf16, tag="wtmp2", name=f"br_{name}")
                nc.vector.tensor_copy(out=br[:], in_=br_f[:])
                bp = psz.tile([P, dcols], f32, space="PSUM", tag="zps", name=f"bp_{name}")
                nc.tensor.matmul(out=bp[:, :], lhsT=ones1[:, :], rhs=br[:, :],
                                 start=True, stop=True)
                bt = const.tile([P, dcols], f32, name=f"bt_{name}")
                nc.vector.tensor_copy(out=bt[:], in_=bp[:, :])
                b_bc[name] = bt

            wm_sb = const.tile([P, 1], f32)
            nc.sync.dma_start(out=wm_sb[:], in_=Wm_in[:, :])
            bm_sb = const.tile([1, 1], f32)
            nc.sync.dma_start(out=bm_sb[:], in_=bm_in[:, None])
            invc_sb = const.tile([1, N_GRAPHS], f32)
            nc.sync.dma_start(out=invc_sb[:], in_=invc_in[None, :])

            h4 = const.tile([P, NPC_PAD], bf16)   # node-major H4 [n, f] per block

            # ---- layer 1 dense: Z1 = X @ W1 -------------------------------
            for t in range(NBLK):
                r0 = t * P
                nr = min(P, NPC - r0)
                if nr <= 0:
                    break
                xf = sbuf.tile([P, D_IN], f32, tag="xf", name=f"xf{t}")
                nc.sync.dma_start(out=xf[0:nr, :], in_=x_in[r0:r0 + nr, :])
                xb = sbuf.tile([P, D_IN], bf16, tag="xb", name=f"xb{t}")
                nc.vector.tensor_copy(out=xb[0:nr, :], in_=xf[0:nr, :])
                xt = sbuf.tile([P, 3 * P], bf16, tag="xt", name=f"xt{t}")
                for c in range(3):
                    wc = CW[c]
                    tp = psp.tile([P, P], bf16, space="PSUM", tag="pp", name=f"tp{t}_{c}")
                    nc.tensor.transpose(out=tp[0:wc, 0:nr],
                                        in_=xb[0:nr, c * P:c * P + wc],
                                        identity=ident[0:nr, 0:nr])
                    nc.vector.tensor_copy(out=xt[0:wc, c * P:c * P + nr],
                                          in_=tp[0:wc, 0:nr])
                zps = psz.tile([P, D_HID], f32, space="PSUM", tag="zps", name=f"zp1_{t}")
                for c in range(3):
                    nc.tensor.matmul(out=zps[0:nr, :],
                                     lhsT=xt[0:CW[c], c * P:c * P + nr],
                                     rhs=w_sb["W1"][c][0:CW[c], :],
                                     start=(c == 0), stop=(c == 2))
                zsb = sbuf.tile([P, D_PAD], bf16, tag="zsb", name=f"zs1_{t}")
                nc.vector.tensor_copy(out=zsb[0:nr, 0:D_HID], in_=zps[0:nr, :])
                nc.sync.dma_start(out=z_shard[1][r0:r0 + nr, 0:D_HID],
                                  in_=zsb[0:nr, 0:D_HID])

            # ---- aggregation layers ---------------------------------------
            reg_cache = {}

            def nreg(v):
                if v not in reg_cache:
                    reg_cache[v] = v  # plain int; bass to_reg per call
                return reg_cache[v]

            def aggregate_layer(layer):
                width = D_HID if layer < 3 else D_OUT
                gwidth = D_PAD if layer < 3 else D_OUT
                zf = z_full[layer]
                nc.gpsimd.collective_compute(
                    "AllGather", mybir.AluOpType.bypass,
                    replica_groups=[list(range(N_CORES))],
                    ins=[z_shard[layer][:, :].opt()],
                    outs=[zf[:, :].opt()])

                gather_of = {}
                for chv, t0, nt in gathers:
                    for t in range(t0, t0 + nt):
                        gather_of[t] = (chv, t0, nt)
                issued = {}

                def ensure_gather(t):
                    chv, t0, nt = gather_of[t]
                    if t0 in issued:
                        return issued[t0], t0
                    msg = sbuf.tile([P, MAX_T, gwidth], bf16, tag="msg",
                                    name=f"msg{layer}_{t0}")
                    nc.gpsimd.dma_gather(
                        out_ap=msg[:, 0:nt, :],
                        in_ap=zf[chv * CHUNK:(chv + 1) * CHUNK, :],
                        idxs_ap=idx_sb[:, t0 * 8:(t0 + nt) * 8],
                        num_idxs=nt * P,
                        num_idxs_reg=nt * P,
                        elem_size=gwidth,
                        elem_step=gwidth)
                    issued[t0] = msg
                    return msg, t0

                for g in range(n_sg):
                    acc = {}
                    for chv, bl, t0, nt in sg_segments[g]:
                        ps = psum.tile([P, D_HID], f32, space="PSUM", tag="seg",
                                       name=f"ps{layer}_{g}_{chv}_{bl}")
                        for t in range(t0, t0 + nt):
                            msg, g0 = ensure_gather(t)
                            oh = sbuf.tile([P, P], bf16, tag="oh",
                                           name=f"oh{layer}_{t}")
                            nc.vector.tensor_scalar(
                                out=oh[:], in0=iotas[0][:],
                                scalar1=doff_sb[:, t:t + 1],
                                scalar2=norm_sb[:, t:t + 1],
                                op0=mybir.AluOpType.is_equal,
                                op1=mybir.AluOpType.mult)
                            nc.tensor.matmul(
                                out=ps[:, 0:width],
                                lhsT=oh[:],
                                rhs=msg[:, t - g0, 0:width],
                                start=(t == t0), stop=(t == t0 + nt - 1))
                        if bl not in acc:
                            a = accp.tile([P, width], f32, tag="acc",
                                          name=f"ac{layer}_{g}_{bl}")
                            acc[bl] = a
                            nc.vector.tensor_copy(out=a[:, :], in_=ps[:, 0:width])
                        else:
                            nc.vector.tensor_add(out=acc[bl][:, :],
                                                 in0=acc[bl][:, :],
                                                 in1=ps[:, 0:width])
                    for bl in sorted(acc):
                        a = acc[bl]
                        bt = b_bc[f"b{layer}"]
                        nc.vector.tensor_add(out=a[:, :], in0=a[:, :], in1=bt[:, 0:width])
                        if layer < 3:
                            h = sbuf.tile([P, width], bf16, tag="h",
                                          name=f"h{layer}_{g}_{bl}")
                            nc.scalar.activation(
                                out=h[:, :], in_=a[:, :],
                                func=mybir.ActivationFunctionType.Relu)
                            ht = sbuf.tile([P, 3 * P], bf16, tag="ht",
                                           name=f"ht{layer}_{g}_{bl}")
                            for c in range(3):
                                wc = CW[c]
                                tp = psp.tile([P, P], bf16, space="PSUM", tag="pp",
                                              name=f"tr{layer}_{g}_{bl}_{c}")
                                nc.tensor.transpose(out=tp[0:wc, :],
                                                    in_=h[:, c * P:c * P + wc],
                                                    identity=ident[:])
                                nc.vector.tensor_copy(out=ht[0:wc, c * P:(c + 1) * P],
                                                      in_=tp[0:wc, :])
                            nl = layer + 1
                            dcols = D_HID if nl < 3 else D_OUT
                            zps = psz.tile([P, D_HID], f32, space="PSUM", tag="zps",
                                           name=f"zp{nl}_{g}_{bl}")
                            for c in range(3):
                                nc.tensor.matmul(
                                    out=zps[:, 0:dcols],
                                    lhsT=ht[0:CW[c], c * P:(c + 1) * P],
                                    rhs=w_sb[f"W{nl}"][c][0:CW[c], :],
                                    start=(c == 0), stop=(c == 2))
                            zsb = sbuf.tile([P, D_PAD], bf16, tag="zsb",
                                            name=f"zs{nl}_{g}_{bl}")
                            nc.vector.tensor_copy(out=zsb[:, 0:dcols],
                                                  in_=zps[:, 0:dcols])
                            nc.sync.dma_start(
                                out=z_shard[nl][bl * P:(bl + 1) * P, 0:dcols],
                                in_=zsb[:, 0:dcols])
                        else:
                            nc.scalar.activation(
                                out=h4[:, bl * P:(bl + 1) * P], in_=a[:, :],
                                func=mybir.ActivationFunctionType.Relu)

            aggregate_layer(1)
            aggregate_layer(2)
            aggregate_layer(3)

            # ---- global mean pool + readout (SPMD-uniform) ----------------
            pool_ps = [psp.tile([P, P], f32, space="PSUM", tag="pp",
                                name=f"poolps{k}") for k in range(4)]
            for bl in range(NBLK):
                for k in range(4):
                    ohg = sbuf.tile([P, P], bf16, tag="oh", name=f"ohg{bl}_{k}")
                    nc.vector.tensor_scalar(
                        out=ohg[:], in0=iotas[k][:],
                        scalar1=batf_sb[:, bl:bl + 1], scalar2=None,
                        op0=mybir.AluOpType.is_equal)
                    nc.tensor.matmul(out=pool_ps[k][:, :], lhsT=ohg[:],
                                     rhs=h4[:, bl * P:(bl + 1) * P],
                                     start=(bl == 0), stop=(bl == NBLK - 1))
            pooledT = const.tile([P, N_GRAPHS], f32)
            for k in range(4):
                pk = sbuf.tile([P, P], bf16, tag="h", name=f"pk{k}")
                nc.vector.tensor_copy(out=pk[:], in_=pool_ps[k][:, :])
                tp = psp.tile([P, P], bf16, space="PSUM", tag="pp",
                              name=f"ptp{k}")
                nc.tensor.transpose(out=tp[:, 0:P], in_=pk[:], identity=ident[:])
                nc.vector.tensor_copy(out=pooledT[:, k * P:(k + 1) * P],
                                      in_=tp[:, 0:P])
            zr_ps = psz.tile([P, N_GRAPHS], f32, space="PSUM", tag="zps", name="zrps")
            nc.tensor.matmul(out=zr_ps[0:1, 0:N_GRAPHS], lhsT=wm_sb[:, :],
                             rhs=pooledT[:, :], start=True, stop=True)
            zr = const.tile([1, N_GRAPHS], f32)
            nc.vector.tensor_copy(out=zr[:], in_=zr_ps[0:1, 0:N_GRAPHS])
            nc.gpsimd.dma_start(out=ar_in[:, :], in_=zr[:])
            nc.gpsimd.collective_compute(
                "AllReduce", mybir.AluOpType.add,
                replica_groups=[list(range(N_CORES))],
                ins=[ar_in[:, :].opt()], outs=[ar_out[:, :].opt()])
            zag = const.tile([1, N_GRAPHS], f32)
            nc.gpsimd.dma_start(out=zag[:], in_=ar_out[:, :])
            nc.vector.tensor_mul(out=zag[:], in0=zag[:], in1=invc_sb[:])
            nc.vector.tensor_scalar(out=zag[:], in0=zag[:],
                                    scalar1=bm_sb[0:1, 0:1], scalar2=None,
                                    op0=mybir.AluOpType.add)
            nc.gpsimd.dma_start(out=out_ext[None, :], in_=zag[:])

    nc.compile()
    return nc


# ---------------------------------------------------------------------------
# entry point
# ---------------------------------------------------------------------------

_CACHE = {}


def kernel(**inputs):
    from concourse import bass_utils

    x = np.ascontiguousarray(np.asarray(inputs["x"], np.float32))
    ei = np.asarray(inputs["edge_index"], np.int64)
    batch = np.asarray(inputs["batch"], np.int64)

    key = (ei.tobytes(), batch.tobytes())
    if "k" not in _CACHE or _CACHE.get("key") != key:
        core_meta, gathers, sg_segments, total_tiles, invcnt = preprocess(ei, batch)
        nc = build_graph(gathers, sg_segments, total_tiles)
        _CACHE.update(k=nc, key=key, core_meta=core_meta, invcnt=invcnt)
    nc = _CACHE["k"]
    core_meta = _CACHE["core_meta"]
    invcnt = _CACHE["invcnt"]

    in_maps = []
    for c in range(N_CORES):
        m = core_meta[c]
        in_maps.append({
            "x": x[c * NPC:(c + 1) * NPC],
            "W1": np.asarray(inputs["W1"], np.float32),
            "W2": np.asarray(inputs["W2"], np.float32),
            "W3": np.asarray(inputs["W3"], np.float32),
            "b1": np.asarray(inputs["b1"], np.float32),
            "b2": np.asarray(inputs["b2"], np.float32),
            "b3": np.asarray(inputs["b3"], np.float32),
            "Wm": np.asarray(inputs["Wm"], np.float32),
            "bm": np.asarray(inputs["bm"], np.float32),
            "m_idx": m["idx"],
            "m_dstoff": m["dstoff"],
            "m_norm": m["norm"],
            "m_batchf": m["batchf"],
            "m_invcnt": invcnt,
        })

    res = bass_utils.run_bass_kernel_spmd(
        nc, in_maps, core_ids=list(range(N_CORES)))
    return np.asarray(res.results[0]["out"], np.float32)
